# revision 5
# baseline (speedup 1.0000x reference)
"""Two-layer GraphSAGE encoder on 8 Trainium2 NeuronCores.

Math (per reference):
  h = L2norm(mean_agg(x) @ W1_l.T + x @ W1_r.T + b1)
  h = BN(relu(h)) (batch stats)        [dropout eval = identity]
  out = L2norm(mean_agg(h) @ W2_l.T + h @ W2_r.T + b2)

Distribution: destination nodes sharded across 8 cores (6272 nodes each,
padded to 50176). Gather sources (x, then the BN'd h) are replicated in
each core's HBM; layer-1 output is AllGathered between layers. BN batch
stats are AllReduced.

Aggregation strategy: edges are packed on the host into chunks of 128
"slots". Each chunk targets one 128-node destination window; a shipped
[128 slots x 128 cols] bf16 "seg" matrix (one-hot scaled by 1/deg) turns
TensorE matmuls into the segment-mean:  psum[f, node] += msg.T @ seg.
Messages are fetched by dma_gather (SWDGE) with int16 indices, so sources
are split at row 32768 into lo/hi call streams.
"""

import math
import numpy as np
import ml_dtypes

import concourse.bass as bass
import concourse.bacc as bacc
import concourse.tile as tile
from concourse import mybir, library_config
from concourse import bass_utils

BF16 = ml_dtypes.bfloat16

# ---------------- problem constants (hardcoded per contract) ----------------
N = 50000
E = 800000
F = 128
NCORES = 8
BN_EPS = 1e-5
NORM_EPS = 1e-12

WIN = 128                    # dst nodes per window (= dense block)
CHUNK = 128                  # edge slots per chunk (= matmul K)
CALL_CHUNKS = 16             # chunks per dma_gather call (2048 idxs)
LO_END = 32768               # int16 gather-index boundary

GROUP_WIN = 4                            # windows per PSUM tile (512 cols)


def _recompute_dims():
    """Derive sharding dims from N/NCORES/WIN (call after patching globals)."""
    global NBLK, PER_CORE, NPAD, NGROUP
    NBLK = math.ceil(N / NCORES / WIN)
    PER_CORE = NBLK * WIN
    NPAD = PER_CORE * NCORES
    NGROUP = math.ceil(NBLK / GROUP_WIN)


_recompute_dims()


# ============================ host-side packing ============================

def _pack_schedule(src, dst):
    """Build the common chunk schedule + per-core tensors.

    Returns dict with:
      K: [NBLK, 2] int, chunks per (window, side) — common across cores
      n_calls: [2] calls per side
      call_valid: list of per-call num_idxs_reg (common)
      per_core: list of dicts with idx_lo, idx_hi, seg, xT-less data
    """
    src = np.asarray(src, np.int64)
    dst = np.asarray(dst, np.int64)

    # per-core edge sets
    cores = []
    for c in range(NCORES):
        lo_n, hi_n = c * PER_CORE, min((c + 1) * PER_CORE, N)
        sel = (dst >= lo_n) & (dst < hi_n)
        s = src[sel]
        dl = dst[sel] - lo_n
        cnt = np.bincount(dl, minlength=PER_CORE)
        inv = (1.0 / np.maximum(cnt, 1)).astype(np.float32)
        w = dl // WIN
        side = (s >= LO_END).astype(np.int64)
        cores.append(dict(s=s, dl=dl, w=w, side=side, inv=inv))

    # common chunk counts per (window, side)
    K = np.zeros((NBLK, 2), np.int64)
    for c in range(NCORES):
        e = cores[c]
        for sd in (0, 1):
            m = e['side'] == sd
            counts = np.bincount(e['w'][m], minlength=NBLK)
            K[:, sd] = np.maximum(K[:, sd], (counts + CHUNK - 1) // CHUNK)
    # every window needs >=1 chunk so its psum columns get written
    empty = (K.sum(axis=1) == 0)
    K[empty, 0] = 1

    nchunk = int(K.sum())
    # global chunk base per window; side-stream position of each chunk
    win_base = np.zeros(NBLK, np.int64)
    win_base[1:] = np.cumsum(K.sum(axis=1))[:-1]
    stream_pos = np.zeros((NBLK, 2), np.int64)   # first stream index per (w, side)
    tot = [0, 0]
    for w in range(NBLK):
        for sd in (0, 1):
            stream_pos[w, sd] = tot[sd]
            tot[sd] += K[w, sd]
    side_chunks = tot                            # total chunks per side
    n_calls = [(side_chunks[sd] + CALL_CHUNKS - 1) // CALL_CHUNKS for sd in (0, 1)]
    call_valid = []
    for sd in (0, 1):
        v = []
        for k in range(n_calls[sd]):
            real = min(CALL_CHUNKS, side_chunks[sd] - k * CALL_CHUNKS)
            v.append(real * CHUNK)
        call_valid.append(v)

    per_core = []
    for c in range(NCORES):
        e = cores[c]
        ns = len(e['s'])
        # order edges by (window, side, anything)
        order = np.lexsort((e['s'], e['side'], e['w']))
        ws = e['w'][order]
        sides = e['side'][order]
        ss = e['s'][order]
        dls = e['dl'][order]
        # rank within (w, side) group
        key = ws * 2 + sides
        if ns > 0:
            uniq, start_idx = np.unique(key, return_index=True)
            group_start = np.zeros(ns, np.int64)
            group_start[start_idx] = start_idx
            group_start = np.maximum.accumulate(group_start)
            rank = np.arange(ns) - group_start
        else:
            rank = np.zeros(0, np.int64)
        chunk_in_side = rank // CHUNK
        slot = rank % CHUNK
        gchunk = win_base[ws] + np.where(sides == 0, 0, K[ws, 0]) + chunk_in_side
        spos = stream_pos[ws, sides] + chunk_in_side

        # seg matrix [128, nchunk*128] (values inv_cnt at (slot, gchunk*128+col))
        seg = np.zeros((CHUNK, nchunk * WIN), np.float32)
        col = dls - ws * WIN
        seg[slot, gchunk * WIN + col] = e['inv'][dls]

        # gather idx per side stream: [n_calls*CALL_CHUNKS*CHUNK] int16
        idxs = []
        for sd in (0, 1):
            arr = np.full(n_calls[sd] * CALL_CHUNKS * CHUNK, -1, np.int64)
            arr[:side_chunks[sd] * CHUNK] = 0     # interior pads gather row 0
            m = sides == sd
            flat = spos[m] * CHUNK + slot[m]
            rebased = ss[m] - (LO_END if sd else 0)
            arr[flat] = rebased
            idxs.append(arr)
        per_core.append(dict(seg=seg, idx=idxs))

    return dict(K=K, nchunk=nchunk, n_calls=n_calls, call_valid=call_valid,
                side_chunks=side_chunks, per_core=per_core)


def _wrap_idx(flat, n_calls):
    """[n_calls*2048] -> [128, n_calls*128] int16 in the dma_gather layout:
    per call, idx i sits at partition i%16, col i//16, replicated x8."""
    out = np.zeros((128, n_calls * CALL_CHUNKS * CHUNK // 16), np.int16)
    per = CALL_CHUNKS * CHUNK
    cols = per // 16
    for k in range(n_calls):
        blk = flat[k * per:(k + 1) * per].reshape(cols, 16).T.astype(np.int16)
        out[:, k * cols:(k + 1) * cols] = np.tile(blk, (8, 1))
    return out


# ============================ kernel builder ============================

def _build_nc(sched):
    K = sched['K']
    nchunk = sched['nchunk']
    n_calls = sched['n_calls']
    call_valid = sched['call_valid']

    nc = bacc.Bacc("TRN2", target_bir_lowering=False, debug=False,
                   num_devices=NCORES, num_swdge_queues=4)
    dt = mybir.dt
    f32, bf16, i16 = dt.float32, dt.bfloat16, dt.int16

    x_d = nc.dram_tensor("x", [N, F], bf16, kind="ExternalInput")
    xT_d = nc.dram_tensor("xT", [F, PER_CORE], bf16, kind="ExternalInput")
    seg_d = nc.dram_tensor("seg", [CHUNK, nchunk * WIN], bf16, kind="ExternalInput")
    idx_d = [None, None]
    for sd in (0, 1):
        if n_calls[sd]:
            idx_d[sd] = nc.dram_tensor(f"idx{sd}", [128, n_calls[sd] * CALL_CHUNKS * CHUNK // 16],
                                       i16, kind="ExternalInput")
    w_d = {}
    for nm in ("W1lT", "W1rT", "W2lT", "W2rT"):
        w_d[nm] = nc.dram_tensor(nm, [F, F], bf16, kind="ExternalInput")
    row_d = {}
    for nm in ("b1", "b2", "gamma", "beta", "ones"):
        row_d[nm] = nc.dram_tensor(nm, [1, F], f32, kind="ExternalInput")
    mask_d = nc.dram_tensor("mask", [CHUNK, NBLK], bf16, kind="ExternalInput")
    ident_d = nc.dram_tensor("ident", [128, 128], bf16, kind="ExternalInput")
    out_d = nc.dram_tensor("out", [PER_CORE, F], f32, kind="ExternalOutput")

    groups = [list(range(NCORES))]
    qctr = [0]

    with tile.TileContext(nc) as tc:
        with tc.tile_pool(name="const", bufs=1) as constp, \
             tc.tile_pool(name="big", bufs=1) as bigp, \
             tc.tile_pool(name="msg", bufs=3) as msgp, \
             tc.tile_pool(name="segp", bufs=3) as segp, \
             tc.tile_pool(name="work", bufs=2) as workp, \
             tc.tile_pool(name="small", bufs=2) as smallp, \
             tc.tile_pool(name="agg_ps", bufs=2, space="PSUM") as aggps, \
             tc.tile_pool(name="h_ps", bufs=2, space="PSUM") as hps, \
             tc.tile_pool(name="st_ps", bufs=1, space="PSUM") as stps, \
             tc.tile_pool(name="dram", bufs=1, space="DRAM") as dramp:

            nc.gpsimd.load_library(library_config.mlp)

            # ---- resident tensors ----
            W = {}
            for nm in ("W1lT", "W1rT", "W2lT", "W2rT"):
                t = constp.tile([F, F], bf16, name=f"t_{nm}")
                nc.sync.dma_start(t[:], w_d[nm][:])
                W[nm] = t
            R = {}
            for nm in ("b1", "b2", "gamma", "beta", "ones"):
                t = constp.tile([1, F], f32, name=f"t_{nm}")
                nc.sync.dma_start(t[:], row_d[nm][:])
                R[nm] = t
            mask_t = constp.tile([CHUNK, NBLK], bf16)
            nc.sync.dma_start(mask_t[:], mask_d[:])
            ident_t = constp.tile([128, 128], bf16)
            nc.sync.dma_start(ident_t[:], ident_d[:])
            xT_t = bigp.tile([F, PER_CORE], bf16)
            nc.sync.dma_start(xT_t[:], xT_d[:])
            idx_t = [None, None]
            for sd in (0, 1):
                if n_calls[sd]:
                    idx_t[sd] = bigp.tile([128, n_calls[sd] * CALL_CHUNKS * CHUNK // 16],
                                          i16, name=f"idx_t{sd}")
                    nc.sync.dma_start(idx_t[sd][:], idx_d[sd][:])

            meanT = [bigp.tile([F, PER_CORE], bf16, name=f"meanT{l}") for l in (0, 1)]
            h_all = bigp.tile([CHUNK, NBLK * F], bf16)       # relu(h1) blocks [node, h]
            hT_all = bigp.tile([F, PER_CORE], bf16)          # bn(h1)^T for layer-2 dense
            abc = [bigp.tile([128, F], f32, name=f"bn_bc{i}") for i in (0, 1)]

            # DRAM bounce buffers
            st_b = dramp.tile([1, 2 * F], f32)
            st_r = dramp.tile([1, 2 * F], f32)
            h_shard = dramp.tile([PER_CORE, F], bf16)
            h_full = dramp.tile([NPAD, F], bf16, addr_space="Shared")

            def aggregate(layer):
                """Chunked seg-matmul aggregation -> meanT[layer]."""
                if layer == 0:
                    base_lo = x_d[0:LO_END, :]
                    base_hi = x_d[LO_END:N, :]
                else:
                    base_lo = h_full[0:LO_END, :]
                    base_hi = h_full[LO_END:NPAD, :]
                bases = (base_lo, base_hi)
                msg_tiles = [{}, {}]

                def get_call(sd, call):
                    if call not in msg_tiles[sd]:
                        t = msgp.tile([CHUNK, CALL_CHUNKS, F], bf16, tag=f"msg{sd}",
                                      name=f"msg_l{layer}_s{sd}_c{call}")
                        cols = CALL_CHUNKS * CHUNK // 16
                        nc.gpsimd.dma_gather(
                            t[:], bases[sd],
                            idx_t[sd][:, call * cols:(call + 1) * cols],
                            CALL_CHUNKS * CHUNK, call_valid[sd][call], F,
                            single_packet=False, queue_num=qctr[0] % 4)
                        qctr[0] += 1
                        msg_tiles[sd][call] = t
                    return msg_tiles[sd][call]

                gchunk = 0
                spos = [0, 0]
                for g in range(NGROUP):
                    wlo = g * GROUP_WIN
                    whi = min(wlo + GROUP_WIN, NBLK)
                    ncols = (whi - wlo) * WIN
                    ps = aggps.tile([F, GROUP_WIN * WIN], f32, tag="agg",
                                    name=f"agg_l{layer}_g{g}", space="PSUM")
                    for w in range(wlo, whi):
                        seg_w = segp.tile([CHUNK, int(K[w].sum()) * WIN], bf16, tag="seg",
                                          name=f"seg_l{layer}_w{w}")
                        nc.sync.dma_start(
                            seg_w[:], seg_d[:, gchunk * WIN:(gchunk + int(K[w].sum())) * WIN])
                        jw = 0
                        for sd in (0, 1):
                            for j in range(int(K[w, sd])):
                                call, pos = divmod(spos[sd], CALL_CHUNKS)
                                mt = get_call(sd, call)
                                nc.tensor.matmul(
                                    out=ps[:, (w - wlo) * WIN:(w - wlo + 1) * WIN],
                                    lhsT=mt[:, pos, :],
                                    rhs=seg_w[:, jw * WIN:(jw + 1) * WIN],
                                    start=(jw == 0), stop=(jw == int(K[w].sum()) - 1))
                                spos[sd] += 1
                                jw += 1
                                gchunk += 1
                    nc.vector.tensor_copy(
                        meanT[layer][:, wlo * WIN:wlo * WIN + ncols], ps[:, :ncols])

            def dense_block(layer, b, ps_sum=None, ps_ssq=None):
                """Dense matmuls + L2 norm (+ relu/stats for layer 0) for block b."""
                mT = meanT[layer][:, b * WIN:(b + 1) * WIN]
                if layer == 0:
                    lT, rT, brow = W["W1lT"], W["W1rT"], R["b1"]
                    xTb = xT_t[:, b * WIN:(b + 1) * WIN]
                else:
                    lT, rT, brow = W["W2lT"], W["W2rT"], R["b2"]
                    xTb = hT_all[:, b * WIN:(b + 1) * WIN]
                ph = hps.tile([CHUNK, F], f32, tag="h", name=f"h_l{layer}_b{b}",
                              space="PSUM")
                nc.tensor.matmul(out=ph[:], lhsT=mT, rhs=lT[:], start=True, stop=False)
                nc.tensor.matmul(out=ph[:], lhsT=xTb, rhs=rT[:], start=False, stop=False)
                nc.tensor.matmul(out=ph[:], lhsT=R["ones"][:], rhs=brow[:],
                                 start=False, stop=True)
                # L2 norm over rows
                sq = workp.tile([CHUNK, F], f32, tag="sq")
                ssum = smallp.tile([CHUNK, 1], f32, tag="ssum")
                nc.scalar.activation(sq[:], ph[:], mybir.ActivationFunctionType.Square,
                                     accum_out=ssum[:])
                nrm = smallp.tile([CHUNK, 1], f32, tag="nrm")
                nc.scalar.sqrt(nrm[:], ssum[:])
                nc.vector.tensor_scalar_max(nrm[:], nrm[:], NORM_EPS)
                rinv = smallp.tile([CHUNK, 1], f32, tag="rinv")
                nc.vector.reciprocal(rinv[:], nrm[:])
                if layer == 0:
                    hr = h_all[:, b * F:(b + 1) * F]
                    nc.scalar.activation(hr, ph[:], mybir.ActivationFunctionType.Relu,
                                         scale=rinv[:])
                    hsq = workp.tile([CHUNK, F], bf16, tag="hsq")
                    nc.vector.tensor_mul(hsq[:], hr, hr)
                    mcol = mask_t[:, b:b + 1]
                    nc.tensor.matmul(out=ps_sum[:], lhsT=mcol, rhs=hr,
                                     start=(b == 0), stop=(b == NBLK - 1))
                    nc.tensor.matmul(out=ps_ssq[:], lhsT=mcol, rhs=hsq[:],
                                     start=(b == 0), stop=(b == NBLK - 1))
                else:
                    ob = workp.tile([CHUNK, F], f32, tag="out")
                    nc.scalar.activation(ob[:], ph[:], mybir.ActivationFunctionType.Copy,
                                         scale=rinv[:])
                    nc.sync.dma_start(out_d[b * WIN:(b + 1) * WIN, :], ob[:])

            # ================= layer 1 =================
            aggregate(0)
            ps_sum = stps.tile([1, F], f32, name="ps_sum", space="PSUM")
            ps_ssq = stps.tile([1, F], f32, name="ps_ssq", space="PSUM")
            for b in range(NBLK):
                dense_block(0, b, ps_sum, ps_ssq)

            # ---- BN stats allreduce ----
            st = smallp.tile([1, 2 * F], f32, name="st")
            nc.vector.tensor_copy(st[:, 0:F], ps_sum[:])
            nc.vector.tensor_copy(st[:, F:2 * F], ps_ssq[:])
            nc.sync.dma_start(st_b[:], st[:])
            nc.gpsimd.collective_compute(
                "AllReduce", mybir.AluOpType.add, replica_groups=groups,
                ins=[st_b.opt()], outs=[st_r.opt()])
            str_t = smallp.tile([1, 2 * F], f32, name="str_t")
            nc.sync.dma_start(str_t[:], st_r[:])
            # a = gamma / sqrt(var + eps); c = beta - mu * a
            mu = smallp.tile([1, F], f32, name="mu")
            nc.vector.tensor_scalar_mul(mu[:], str_t[:, 0:F], 1.0 / N)
            ex2 = smallp.tile([1, F], f32, name="ex2")
            nc.vector.tensor_scalar_mul(ex2[:], str_t[:, F:2 * F], 1.0 / N)
            var = smallp.tile([1, F], f32, name="var")
            nc.vector.tensor_mul(var[:], mu[:], mu[:])
            nc.vector.tensor_sub(var[:], ex2[:], var[:])
            nc.vector.tensor_scalar_add(var[:], var[:], BN_EPS)
            sd_t = smallp.tile([1, F], f32, name="sd_t")
            nc.scalar.sqrt(sd_t[:], var[:])
            rsd = smallp.tile([1, F], f32, name="rsd")
            nc.vector.reciprocal(rsd[:], sd_t[:])
            a_row = smallp.tile([1, F], f32, name="a_row")
            nc.vector.tensor_mul(a_row[:], R["gamma"][:], rsd[:])
            c_row = smallp.tile([1, F], f32, name="c_row")
            nc.vector.tensor_mul(c_row[:], mu[:], a_row[:])
            nc.vector.tensor_sub(c_row[:], R["beta"][:], c_row[:])
            # broadcast to [128, F] via K=1 matmuls
            for i, rowt in enumerate((a_row, c_row)):
                pbc = hps.tile([128, F], f32, tag="h", name=f"bc{i}", space="PSUM")
                nc.tensor.matmul(out=pbc[:], lhsT=R["ones"][:], rhs=rowt[:],
                                 start=True, stop=True)
                nc.vector.tensor_copy(abc[i][:], pbc[:])

            # ---- apply BN, build h_shard + hT_all ----
            for b in range(NBLK):
                hr = h_all[:, b * F:(b + 1) * F]
                hb = workp.tile([CHUNK, F], bf16, tag="hb", name=f"hb{b}")
                nc.vector.tensor_mul(hb[:], hr, abc[0][:])
                nc.vector.tensor_add(hb[:], hb[:], abc[1][:])
                nc.sync.dma_start(h_shard[b * WIN:(b + 1) * WIN, :], hb[:])
                pt = hps.tile([128, F], bf16, tag="ht", name=f"ht{b}", space="PSUM")
                nc.tensor.transpose(out=pt[:], in_=hb[:], identity=ident_t[:])
                nc.vector.tensor_copy(hT_all[:, b * WIN:(b + 1) * WIN], pt[:])

            nc.gpsimd.collective_compute(
                "AllGather", mybir.AluOpType.bypass, replica_groups=groups,
                ins=[h_shard.opt()], outs=[h_full.opt()])

            # ================= layer 2 =================
            aggregate(1)
            for b in range(NBLK):
                dense_block(1, b)

    nc.compile()
    return nc


# ============================ top-level entry ============================

_CACHE = {}


def kernel(x, edge_index, W1_l, W1_r, b1, gamma, beta, W2_l, W2_r, b2):
    x = np.asarray(x, np.float32)
    src = np.asarray(edge_index[0], np.int64)
    dst = np.asarray(edge_index[1], np.int64)

    sched = _pack_schedule(src, dst)
    nc = _build_nc(sched)

    x_bf = x.astype(BF16)
    ident = np.eye(128, dtype=np.float32).astype(BF16)
    ones = np.ones((1, F), np.float32)

    def row(v):
        return np.asarray(v, np.float32).reshape(1, F)

    in_maps = []
    for c in range(NCORES):
        pc = sched['per_core'][c]
        lo_n = c * PER_CORE
        hi_n = max(min(lo_n + PER_CORE, N), lo_n)
        xT = np.zeros((F, PER_CORE), np.float32)
        xT[:, :hi_n - lo_n] = x[lo_n:hi_n].T
        mask = np.zeros((CHUNK, NBLK), np.float32)
        flat = np.arange(PER_CORE) + lo_n < N
        mask[:, :] = flat.reshape(NBLK, CHUNK).T
        m = dict(
            x=x_bf,
            xT=xT.astype(BF16),
            seg=pc['seg'].astype(BF16),
            W1lT=np.ascontiguousarray(np.asarray(W1_l, np.float32).T).astype(BF16),
            W1rT=np.ascontiguousarray(np.asarray(W1_r, np.float32).T).astype(BF16),
            W2lT=np.ascontiguousarray(np.asarray(W2_l, np.float32).T).astype(BF16),
            W2rT=np.ascontiguousarray(np.asarray(W2_r, np.float32).T).astype(BF16),
            b1=row(b1), b2=row(b2), gamma=row(gamma), beta=row(beta),
            ones=ones, mask=mask.astype(BF16), ident=ident,
        )
        for sd in (0, 1):
            if sched['n_calls'][sd]:
                m[f"idx{sd}"] = _wrap_idx(pc['idx'][sd], sched['n_calls'][sd])
        in_maps.append(m)

    r = bass_utils.run_bass_kernel_spmd(nc, in_maps, core_ids=list(range(NCORES)),
                                        trace=False)
    global _last_result
    _last_result = r
    out = np.concatenate([r.results[c]["out"] for c in range(NCORES)], axis=0)
    return out[:N].astype(np.float32)


_last_result = None


# revision 10
# speedup vs baseline: 1.0760x; 1.0760x over previous
"""Two-layer GraphSAGE encoder on 8 Trainium2 NeuronCores.

Math (per reference):
  h = L2norm(mean_agg(x) @ W1_l.T + x @ W1_r.T + b1)
  h = BN(relu(h)) (batch stats)        [dropout eval = identity]
  out = L2norm(mean_agg(h) @ W2_l.T + h @ W2_r.T + b2)

Distribution: destination nodes sharded across 8 cores (6272 nodes each,
padded to 50176). Gather sources (x, then the BN'd h) are replicated in
each core's HBM; layer-1 output is AllGathered between layers. BN batch
stats are AllReduced.

Aggregation strategy: edges are packed on the host into chunks of 128
"slots". Each chunk targets one 128-node destination window; a shipped
[128 slots x 128 cols] bf16 "seg" matrix (one-hot scaled by 1/deg) turns
TensorE matmuls into the segment-mean:  psum[f, node] += msg.T @ seg.
Messages are fetched by dma_gather (SWDGE) with int16 indices, so sources
are split at row 32768 into lo/hi call streams.
"""

import math
import numpy as np
import ml_dtypes

import concourse.bass as bass
import concourse.bacc as bacc
import concourse.tile as tile
from concourse import mybir, library_config
from concourse import bass_utils

BF16 = ml_dtypes.bfloat16

# ---------------- problem constants (hardcoded per contract) ----------------
N = 50000
E = 800000
F = 128
NCORES = 8
BN_EPS = 1e-5
NORM_EPS = 1e-12

WIN = 128                    # dst nodes per window (= dense block)
CHUNK = 128                  # edge slots per chunk (= matmul K)
CALL_CHUNKS = 16             # chunks per dma_gather call (2048 idxs)
LO_END = 32768               # int16 gather-index boundary

GROUP_WIN = 4                            # windows per PSUM tile (512 cols)


def _recompute_dims():
    """Derive sharding dims from N/NCORES/WIN (call after patching globals)."""
    global NBLK, PER_CORE, NPAD, NGROUP
    NBLK = math.ceil(N / NCORES / WIN)
    PER_CORE = NBLK * WIN
    NPAD = PER_CORE * NCORES
    NGROUP = math.ceil(NBLK / GROUP_WIN)


_recompute_dims()


# ============================ host-side packing ============================

def _pack_schedule(src, dst):
    """Build the common chunk schedule + per-core tensors.

    Returns dict with:
      K: [NBLK, 2] int, chunks per (window, side) — common across cores
      n_calls: [2] calls per side
      call_valid: list of per-call num_idxs_reg (common)
      per_core: list of dicts with idx_lo, idx_hi, seg, xT-less data
    """
    src = np.asarray(src, np.int64)
    dst = np.asarray(dst, np.int64)

    # per-core edge sets
    cores = []
    for c in range(NCORES):
        lo_n, hi_n = c * PER_CORE, min((c + 1) * PER_CORE, N)
        sel = (dst >= lo_n) & (dst < hi_n)
        s = src[sel]
        dl = dst[sel] - lo_n
        cnt = np.bincount(dl, minlength=PER_CORE)
        inv = (1.0 / np.maximum(cnt, 1)).astype(np.float32)
        w = dl // WIN
        side = (s >= LO_END).astype(np.int64)
        cores.append(dict(s=s, dl=dl, w=w, side=side, inv=inv))

    # common chunk counts per (window, side)
    K = np.zeros((NBLK, 2), np.int64)
    for c in range(NCORES):
        e = cores[c]
        for sd in (0, 1):
            m = e['side'] == sd
            counts = np.bincount(e['w'][m], minlength=NBLK)
            K[:, sd] = np.maximum(K[:, sd], (counts + CHUNK - 1) // CHUNK)
    # every window needs >=1 chunk so its psum columns get written
    empty = (K.sum(axis=1) == 0)
    K[empty, 0] = 1

    nchunk = int(K.sum())
    # global chunk base per window; side-stream position of each chunk
    win_base = np.zeros(NBLK, np.int64)
    win_base[1:] = np.cumsum(K.sum(axis=1))[:-1]
    stream_pos = np.zeros((NBLK, 2), np.int64)   # first stream index per (w, side)
    tot = [0, 0]
    for w in range(NBLK):
        for sd in (0, 1):
            stream_pos[w, sd] = tot[sd]
            tot[sd] += K[w, sd]
    side_chunks = tot                            # total chunks per side
    n_calls = [(side_chunks[sd] + CALL_CHUNKS - 1) // CALL_CHUNKS for sd in (0, 1)]
    call_valid = []
    for sd in (0, 1):
        v = []
        for k in range(n_calls[sd]):
            real = min(CALL_CHUNKS, side_chunks[sd] - k * CALL_CHUNKS)
            v.append(real * CHUNK)
        call_valid.append(v)

    per_core = []
    for c in range(NCORES):
        e = cores[c]
        ns = len(e['s'])
        # order edges by (window, side, anything)
        order = np.lexsort((e['s'], e['side'], e['w']))
        ws = e['w'][order]
        sides = e['side'][order]
        ss = e['s'][order]
        dls = e['dl'][order]
        # rank within (w, side) group
        key = ws * 2 + sides
        if ns > 0:
            uniq, start_idx = np.unique(key, return_index=True)
            group_start = np.zeros(ns, np.int64)
            group_start[start_idx] = start_idx
            group_start = np.maximum.accumulate(group_start)
            rank = np.arange(ns) - group_start
        else:
            rank = np.zeros(0, np.int64)
        chunk_in_side = rank // CHUNK
        slot = rank % CHUNK
        gchunk = win_base[ws] + np.where(sides == 0, 0, K[ws, 0]) + chunk_in_side
        spos = stream_pos[ws, sides] + chunk_in_side

        # seg matrix [128, nchunk*128] (values inv_cnt at (slot, gchunk*128+col))
        seg = np.zeros((CHUNK, nchunk * WIN), np.float32)
        col = dls - ws * WIN
        seg[slot, gchunk * WIN + col] = e['inv'][dls]

        # gather idx per side stream: [n_calls*CALL_CHUNKS*CHUNK] int16
        idxs = []
        for sd in (0, 1):
            arr = np.full(n_calls[sd] * CALL_CHUNKS * CHUNK, -1, np.int64)
            arr[:side_chunks[sd] * CHUNK] = 0     # interior pads gather row 0
            m = sides == sd
            flat = spos[m] * CHUNK + slot[m]
            rebased = ss[m] - (LO_END if sd else 0)
            arr[flat] = rebased
            idxs.append(arr)
        per_core.append(dict(seg=seg, idx=idxs))

    return dict(K=K, nchunk=nchunk, n_calls=n_calls, call_valid=call_valid,
                side_chunks=side_chunks, per_core=per_core)


def _wrap_idx(flat, n_calls):
    """[n_calls*2048] -> [128, n_calls*128] int16 in the dma_gather layout:
    per call, idx i sits at partition i%16, col i//16, replicated x8."""
    out = np.zeros((128, n_calls * CALL_CHUNKS * CHUNK // 16), np.int16)
    per = CALL_CHUNKS * CHUNK
    cols = per // 16
    for k in range(n_calls):
        blk = flat[k * per:(k + 1) * per].reshape(cols, 16).T.astype(np.int16)
        out[:, k * cols:(k + 1) * cols] = np.tile(blk, (8, 1))
    return out


# ============================ kernel builder ============================

def _build_nc(sched):
    K = sched['K']
    nchunk = sched['nchunk']
    n_calls = sched['n_calls']
    call_valid = sched['call_valid']

    nc = bacc.Bacc("TRN2", target_bir_lowering=False, debug=False,
                   num_devices=NCORES, num_swdge_queues=4)
    dt = mybir.dt
    f32, bf16, i16 = dt.float32, dt.bfloat16, dt.int16

    x_d = nc.dram_tensor("x", [N, F], bf16, kind="ExternalInput")
    xT_d = nc.dram_tensor("xT", [F, PER_CORE], bf16, kind="ExternalInput")
    seg_d = nc.dram_tensor("seg", [CHUNK, nchunk * WIN], bf16, kind="ExternalInput")
    idx_d = [None, None]
    for sd in (0, 1):
        if n_calls[sd]:
            idx_d[sd] = nc.dram_tensor(f"idx{sd}", [128, n_calls[sd] * CALL_CHUNKS * CHUNK // 16],
                                       i16, kind="ExternalInput")
    w_d = {}
    for nm in ("W1lT", "W1rT", "W2lT", "W2rT"):
        w_d[nm] = nc.dram_tensor(nm, [F, F], bf16, kind="ExternalInput")
    row_d = {}
    for nm in ("b1", "b2", "gamma", "beta", "ones"):
        row_d[nm] = nc.dram_tensor(nm, [1, F], f32, kind="ExternalInput")
    mask_d = nc.dram_tensor("mask", [CHUNK, NBLK], bf16, kind="ExternalInput")
    ident_d = nc.dram_tensor("ident", [128, 128], bf16, kind="ExternalInput")
    out_d = nc.dram_tensor("out", [PER_CORE, F], f32, kind="ExternalOutput")

    groups = [list(range(NCORES))]
    qctr = [0]

    with tile.TileContext(nc) as tc:
        with tc.tile_pool(name="const", bufs=1) as constp, \
             tc.tile_pool(name="big", bufs=1) as bigp, \
             tc.tile_pool(name="msg", bufs=3) as msgp, \
             tc.tile_pool(name="segp", bufs=3) as segp, \
             tc.tile_pool(name="work", bufs=2) as workp, \
             tc.tile_pool(name="small", bufs=2) as smallp, \
             tc.tile_pool(name="agg_ps", bufs=2, space="PSUM") as aggps, \
             tc.tile_pool(name="h_ps", bufs=2, space="PSUM") as hps, \
             tc.tile_pool(name="st_ps", bufs=1, space="PSUM") as stps, \
             tc.tile_pool(name="dram", bufs=1, space="DRAM") as dramp:

            nc.gpsimd.load_library(library_config.mlp)

            # ---- resident tensors ----
            W = {}
            for nm in ("W1lT", "W1rT", "W2lT", "W2rT"):
                t = constp.tile([F, F], bf16, name=f"t_{nm}")
                nc.sync.dma_start(t[:], w_d[nm][:])
                W[nm] = t
            R = {}
            for nm in ("b1", "b2", "gamma", "beta", "ones"):
                t = constp.tile([1, F], f32, name=f"t_{nm}")
                nc.sync.dma_start(t[:], row_d[nm][:])
                R[nm] = t
            mask_t = constp.tile([CHUNK, NBLK], bf16)
            nc.sync.dma_start(mask_t[:], mask_d[:])
            ident_t = constp.tile([128, 128], bf16)
            nc.sync.dma_start(ident_t[:], ident_d[:])
            xT_t = bigp.tile([F, PER_CORE], bf16)
            nc.sync.dma_start(xT_t[:], xT_d[:])
            idx_t = [None, None]
            for sd in (0, 1):
                if n_calls[sd]:
                    idx_t[sd] = bigp.tile([128, n_calls[sd] * CALL_CHUNKS * CHUNK // 16],
                                          i16, name=f"idx_t{sd}")
                    nc.sync.dma_start(idx_t[sd][:], idx_d[sd][:])

            meanT = [bigp.tile([F, PER_CORE], bf16, name=f"meanT{l}") for l in (0, 1)]
            h_all = bigp.tile([CHUNK, NBLK * F], bf16)       # relu(h1) blocks [node, h]
            hT_all = bigp.tile([F, PER_CORE], bf16)          # bn(h1)^T for layer-2 dense
            abc = [bigp.tile([128, F], f32, name=f"bn_bc{i}") for i in (0, 1)]

            # DRAM bounce buffers
            st_b = dramp.tile([1, 2 * F], f32)
            st_r = dramp.tile([1, 2 * F], f32)
            h_shard = dramp.tile([PER_CORE, F], bf16)
            h_full = dramp.tile([NPAD, F], bf16, addr_space="Shared")

            def aggregate_group(layer, g, state):
                """Seg-matmul aggregation for psum group g -> meanT[layer] slice."""
                if layer == 0:
                    bases = (x_d[0:LO_END, :], x_d[LO_END:N, :])
                else:
                    bases = (h_full[0:LO_END, :], h_full[LO_END:NPAD, :])
                msg_tiles = state.setdefault('msg', [{}, {}])

                def get_call(sd, call):
                    if call not in msg_tiles[sd]:
                        t = msgp.tile([CHUNK, CALL_CHUNKS, F], bf16, tag=f"msg{sd}",
                                      name=f"msg_l{layer}_s{sd}_c{call}")
                        cols = CALL_CHUNKS * CHUNK // 16
                        nc.gpsimd.dma_gather(
                            t[:], bases[sd],
                            idx_t[sd][:, call * cols:(call + 1) * cols],
                            CALL_CHUNKS * CHUNK, call_valid[sd][call], F,
                            single_packet=False, queue_num=qctr[0] % 4)
                        qctr[0] += 1
                        msg_tiles[sd][call] = t
                    return msg_tiles[sd][call]

                wlo = g * GROUP_WIN
                whi = min(wlo + GROUP_WIN, NBLK)
                ncols = (whi - wlo) * WIN
                ps = aggps.tile([F, GROUP_WIN * WIN], f32, tag="agg",
                                name=f"agg_l{layer}_g{g}", space="PSUM")
                for w in range(wlo, whi):
                    gchunk = state['gchunk']
                    seg_w = segp.tile([CHUNK, int(K[w].sum()) * WIN], bf16, tag="seg",
                                      name=f"seg_l{layer}_w{w}")
                    nc.sync.dma_start(
                        seg_w[:], seg_d[:, gchunk * WIN:(gchunk + int(K[w].sum())) * WIN])
                    jw = 0
                    for sd in (0, 1):
                        for j in range(int(K[w, sd])):
                            call, pos = divmod(state['spos'][sd], CALL_CHUNKS)
                            mt = get_call(sd, call)
                            nc.tensor.matmul(
                                out=ps[:, (w - wlo) * WIN:(w - wlo + 1) * WIN],
                                lhsT=mt[:, pos, :],
                                rhs=seg_w[:, jw * WIN:(jw + 1) * WIN],
                                start=(jw == 0), stop=(jw == int(K[w].sum()) - 1))
                            state['spos'][sd] += 1
                            jw += 1
                            state['gchunk'] += 1
                nc.vector.tensor_copy(
                    meanT[layer][:, wlo * WIN:wlo * WIN + ncols], ps[:, :ncols])

            def dense_block(layer, b, ps_sum=None, ps_ssq=None):
                """Dense matmuls + L2 norm (+ relu/stats for layer 0) for block b."""
                mT = meanT[layer][:, b * WIN:(b + 1) * WIN]
                if layer == 0:
                    lT, rT, brow = W["W1lT"], W["W1rT"], R["b1"]
                    xTb = xT_t[:, b * WIN:(b + 1) * WIN]
                else:
                    lT, rT, brow = W["W2lT"], W["W2rT"], R["b2"]
                    xTb = hT_all[:, b * WIN:(b + 1) * WIN]
                ph = hps.tile([CHUNK, F], f32, tag="h", name=f"h_l{layer}_b{b}",
                              space="PSUM")
                nc.tensor.matmul(out=ph[:], lhsT=mT, rhs=lT[:], start=True, stop=False)
                nc.tensor.matmul(out=ph[:], lhsT=xTb, rhs=rT[:], start=False, stop=False)
                nc.tensor.matmul(out=ph[:], lhsT=R["ones"][:], rhs=brow[:],
                                 start=False, stop=True)
                # L2 norm over rows
                sq = workp.tile([CHUNK, F], f32, tag="sq")
                ssum = smallp.tile([CHUNK, 1], f32, tag="ssum")
                nc.scalar.activation(sq[:], ph[:], mybir.ActivationFunctionType.Square,
                                     accum_out=ssum[:])
                nrm = smallp.tile([CHUNK, 1], f32, tag="nrm")
                nc.scalar.sqrt(nrm[:], ssum[:])
                nc.vector.tensor_scalar_max(nrm[:], nrm[:], NORM_EPS)
                rinv = smallp.tile([CHUNK, 1], f32, tag="rinv")
                nc.vector.reciprocal(rinv[:], nrm[:])
                if layer == 0:
                    hr = h_all[:, b * F:(b + 1) * F]
                    nc.scalar.activation(hr, ph[:], mybir.ActivationFunctionType.Relu,
                                         scale=rinv[:])
                    hsq = workp.tile([CHUNK, F], bf16, tag="hsq")
                    nc.vector.tensor_mul(hsq[:], hr, hr)
                    mcol = mask_t[:, b:b + 1]
                    nc.tensor.matmul(out=ps_sum[:], lhsT=mcol, rhs=hr,
                                     start=(b == 0), stop=(b == NBLK - 1))
                    nc.tensor.matmul(out=ps_ssq[:], lhsT=mcol, rhs=hsq[:],
                                     start=(b == 0), stop=(b == NBLK - 1))
                    # pre-BN transpose (BN applied later, per-partition, in place)
                    pt = hps.tile([128, F], bf16, tag="ht", name=f"ht{b}",
                                  space="PSUM")
                    nc.tensor.transpose(out=pt[:], in_=hr, identity=ident_t[:])
                    nc.vector.tensor_copy(hT_all[:, b * WIN:(b + 1) * WIN], pt[:])
                else:
                    ob = workp.tile([CHUNK, F], f32, tag="out")
                    nc.scalar.activation(ob[:], ph[:], mybir.ActivationFunctionType.Copy,
                                         scale=rinv[:])
                    nc.sync.dma_start(out_d[b * WIN:(b + 1) * WIN, :], ob[:])

            # ================= layer 1 =================
            ps_sum = stps.tile([1, F], f32, name="ps_sum", space="PSUM")
            ps_ssq = stps.tile([1, F], f32, name="ps_ssq", space="PSUM")
            st0 = {'gchunk': 0, 'spos': [0, 0]}
            for g in range(NGROUP):
                aggregate_group(0, g, st0)
                for b in range(g * GROUP_WIN, min((g + 1) * GROUP_WIN, NBLK)):
                    dense_block(0, b, ps_sum, ps_ssq)

            # ---- BN stats allreduce ----
            st = smallp.tile([1, 2 * F], f32, name="st")
            nc.vector.tensor_copy(st[:, 0:F], ps_sum[:])
            nc.vector.tensor_copy(st[:, F:2 * F], ps_ssq[:])
            nc.sync.dma_start(st_b[:], st[:])
            nc.gpsimd.collective_compute(
                "AllReduce", mybir.AluOpType.add, replica_groups=groups,
                ins=[st_b.opt()], outs=[st_r.opt()])
            str_t = smallp.tile([1, 2 * F], f32, name="str_t")
            nc.sync.dma_start(str_t[:], st_r[:])
            # a = gamma / sqrt(var + eps); c = beta - mu * a
            mu = smallp.tile([1, F], f32, name="mu")
            nc.vector.tensor_scalar_mul(mu[:], str_t[:, 0:F], 1.0 / N)
            ex2 = smallp.tile([1, F], f32, name="ex2")
            nc.vector.tensor_scalar_mul(ex2[:], str_t[:, F:2 * F], 1.0 / N)
            var = smallp.tile([1, F], f32, name="var")
            nc.vector.tensor_mul(var[:], mu[:], mu[:])
            nc.vector.tensor_sub(var[:], ex2[:], var[:])
            nc.vector.tensor_scalar_add(var[:], var[:], BN_EPS)
            sd_t = smallp.tile([1, F], f32, name="sd_t")
            nc.scalar.sqrt(sd_t[:], var[:])
            rsd = smallp.tile([1, F], f32, name="rsd")
            nc.vector.reciprocal(rsd[:], sd_t[:])
            a_row = smallp.tile([1, F], f32, name="a_row")
            nc.vector.tensor_mul(a_row[:], R["gamma"][:], rsd[:])
            c_row = smallp.tile([1, F], f32, name="c_row")
            nc.vector.tensor_mul(c_row[:], mu[:], a_row[:])
            nc.vector.tensor_sub(c_row[:], R["beta"][:], c_row[:])
            # broadcast a,c to [128, F] (node-major) and [128,1] (transposed)
            for i, rowt in enumerate((a_row, c_row)):
                pbc = hps.tile([128, F], f32, tag="h", name=f"bc{i}", space="PSUM")
                nc.tensor.matmul(out=pbc[:], lhsT=R["ones"][:], rhs=rowt[:],
                                 start=True, stop=True)
                nc.vector.tensor_copy(abc[i][:], pbc[:])
            cols = []
            for i, rowt in enumerate((a_row, c_row)):
                pcl = hps.tile([128, 1], f32, tag="h", name=f"col{i}", space="PSUM")
                nc.tensor.matmul(out=pcl[:], lhsT=rowt[:], rhs=R["ones"][:, 0:1],
                                 start=True, stop=True)
                ct = smallp.tile([128, 1], f32, name=f"colsb{i}")
                nc.vector.tensor_copy(ct[:], pcl[:])
                cols.append(ct)
            # transposed BN apply (per-partition scalars), one wide op
            nc.vector.tensor_scalar(hT_all[:], hT_all[:], cols[0][:], cols[1][:],
                                    mybir.AluOpType.mult, mybir.AluOpType.add)
            # node-major BN apply + ship shard
            for b in range(NBLK):
                hr = h_all[:, b * F:(b + 1) * F]
                hb = workp.tile([CHUNK, F], bf16, tag="hb", name=f"hb{b}")
                nc.vector.tensor_mul(hb[:], hr, abc[0][:])
                nc.vector.tensor_add(hb[:], hb[:], abc[1][:])
                nc.sync.dma_start(h_shard[b * WIN:(b + 1) * WIN, :], hb[:])

            nc.gpsimd.collective_compute(
                "AllGather", mybir.AluOpType.bypass, replica_groups=groups,
                ins=[h_shard.opt()], outs=[h_full.opt()])

            # ================= layer 2 =================
            st1 = {'gchunk': 0, 'spos': [0, 0]}
            for g in range(NGROUP):
                aggregate_group(1, g, st1)
                for b in range(g * GROUP_WIN, min((g + 1) * GROUP_WIN, NBLK)):
                    dense_block(1, b)

    nc.compile()
    return nc


# ============================ top-level entry ============================

_CACHE = {}


def kernel(x, edge_index, W1_l, W1_r, b1, gamma, beta, W2_l, W2_r, b2):
    x = np.asarray(x, np.float32)
    src = np.asarray(edge_index[0], np.int64)
    dst = np.asarray(edge_index[1], np.int64)

    sched = _pack_schedule(src, dst)
    nc = _build_nc(sched)

    x_bf = x.astype(BF16)
    ident = np.eye(128, dtype=np.float32).astype(BF16)
    ones = np.ones((1, F), np.float32)

    def row(v):
        return np.asarray(v, np.float32).reshape(1, F)

    in_maps = []
    for c in range(NCORES):
        pc = sched['per_core'][c]
        lo_n = c * PER_CORE
        hi_n = max(min(lo_n + PER_CORE, N), lo_n)
        xT = np.zeros((F, PER_CORE), np.float32)
        xT[:, :hi_n - lo_n] = x[lo_n:hi_n].T
        mask = np.zeros((CHUNK, NBLK), np.float32)
        flat = np.arange(PER_CORE) + lo_n < N
        mask[:, :] = flat.reshape(NBLK, CHUNK).T
        m = dict(
            x=x_bf,
            xT=xT.astype(BF16),
            seg=pc['seg'].astype(BF16),
            W1lT=np.ascontiguousarray(np.asarray(W1_l, np.float32).T).astype(BF16),
            W1rT=np.ascontiguousarray(np.asarray(W1_r, np.float32).T).astype(BF16),
            W2lT=np.ascontiguousarray(np.asarray(W2_l, np.float32).T).astype(BF16),
            W2rT=np.ascontiguousarray(np.asarray(W2_r, np.float32).T).astype(BF16),
            b1=row(b1), b2=row(b2), gamma=row(gamma), beta=row(beta),
            ones=ones, mask=mask.astype(BF16), ident=ident,
        )
        for sd in (0, 1):
            if sched['n_calls'][sd]:
                m[f"idx{sd}"] = _wrap_idx(pc['idx'][sd], sched['n_calls'][sd])
        in_maps.append(m)

    r = bass_utils.run_bass_kernel_spmd(nc, in_maps, core_ids=list(range(NCORES)),
                                        trace=False)
    global _last_result
    _last_result = r
    out = np.concatenate([r.results[c]["out"] for c in range(NCORES)], axis=0)
    return out[:N].astype(np.float32)


_last_result = None
